# revision 52
# baseline (speedup 1.0000x reference)
"""Distributed Trainium2 (Bass/Tile) kernel for a causal self-attention block.

Reference computation (per batch b):
    qk = x_eps @ W_eps_attn ; q,k = split(qk) ; vp = v @ W_attn
    q,k = rope(q), rope(k)   (llama-style, 16 heads x 128 dims)
    y   = causal_softmax(q k^T / sqrt(128)) @ vp   (per head)
    v_out     = y @ W_proj
    x_eps_out = x_eps @ W_eps_proj

Sharding over 8 NeuronCores: core = (b, g) = 4-way batch x 2-way head-group
(8 heads per core).  W_eps_attn/W_attn are column-sharded by head; y is
exchanged pair-wise in two chunks (AllGather per 512-token half, pipelined
behind later compute) and W_proj/W_eps_proj are used column-sharded so each
core produces a disjoint half of both outputs (no reduce needed).

All matmuls run in bf16 with fp32 PSUM accumulation.  Activations are
uploaded contraction-major (pre-transposed host-side, numerically identical
to a device-side transpose of the same bf16 data); the rotate-half RoPE
layout, the 1/sqrt(128) q-scale and all weight tiling/permutation are pure
host-side weight re-layout.

Engine balance (v2): RoPE multiplies read the PSUM directly (no DVE
pre-copies), the causal mask is applied in-place on GpSimd via
affine_select, the softmax denominator uses the fast approximate DVE
reciprocal, vp/out PSUM evacuation is split between ScalarE and DVE, and
the x_eps_out projection is interleaved into the second attention half so
TensorE never waits on ScalarE's exp stream.
"""

import sys

sys.path.insert(0, "/opt/trn_rl_repo")

import numpy as np
import ml_dtypes

import concourse.bass as bass
import concourse.mybir as mybir
import concourse.tile as tile
from concourse import bacc
from concourse.bass_utils import run_bass_kernel_spmd

F32 = mybir.dt.float32
BF16 = mybir.dt.bfloat16
BF16_NP = ml_dtypes.bfloat16

B, T, DIM, H, HD = 4, 1024, 2048, 16, 128
NCORES = 8
HL = H // 2          # heads per core (8)
TT = T // 128        # t-tiles (8)
CC = DIM // 128      # contraction chunks (16)
QC = T // 512        # 512-wide q chunks (2)
COLS = DIM // 2      # local column count of each output half (1024)

_COMPILED = None
STAGE_MARKS = []


def _mark(nc, name):
    STAGE_MARKS.append((name, len(nc.inst_map)))


def _build():
    nc = bacc.Bacc(trn_type="TRN2", target_bir_lowering=False, debug=False,
                   num_devices=NCORES)

    # ---- per-core I/O (activations contraction-major, weights pre-packed) ----
    x_in = nc.dram_tensor("xT_bf", [CC, 128, T], BF16, kind="ExternalInput").ap()
    v_in = nc.dram_tensor("vT_bf", [CC, 128, T], BF16, kind="ExternalInput").ap()
    cos_in = nc.dram_tensor("cos_t", [64, T], BF16, kind="ExternalInput").ap()
    sin_in = nc.dram_tensor("sin_t", [64, T], BF16, kind="ExternalInput").ap()
    wqk_in = nc.dram_tensor("w_qk", [8, CC, 128, 256], BF16, kind="ExternalInput").ap()
    wat_in = nc.dram_tensor("w_attn", [CC, 128, COLS], BF16, kind="ExternalInput").ap()
    wpr_in = nc.dram_tensor("w_proj", [CC, 128, COLS], BF16, kind="ExternalInput").ap()
    wep_in = nc.dram_tensor("w_eps_proj", [CC, 128, COLS], BF16,
                            kind="ExternalInput").ap()
    v_out = nc.dram_tensor("v_out", [TT, 128, COLS], BF16, kind="ExternalOutput").ap()
    x_out = nc.dram_tensor("x_out", [TT, 128, COLS], BF16, kind="ExternalOutput").ap()

    # internal DRAM for the chunked pair-wise y exchange
    y_bounce = [nc.dram_tensor(f"y_bounce{qc}", [HL, 128, 512], BF16)
                for qc in range(QC)]
    y_gather = [nc.dram_tensor(f"y_gather{qc}", [2 * HL, 128, 512], BF16)
                for qc in range(QC)]

    with tile.TileContext(nc) as tc:
        with tc.tile_pool(name="persist", bufs=1) as pp, \
             tc.tile_pool(name="wstream", bufs=4) as wsp, \
             tc.tile_pool(name="ptile", bufs=6) as ptp, \
             tc.tile_pool(name="ropetmp", bufs=2) as rtp, \
             tc.tile_pool(name="outcp", bufs=3) as ocp, \
             tc.tile_pool(name="ps", bufs=6, space="PSUM") as psp:
            # PSUM budget: tag "ps" 6 banks (B groups, vp/proj subgroups,
            # score scratch) + tag "pyz" 2 banks (attention accumulators)

            # all-ones stationary operand: the denominator matmul then lands
            # Z replicated on every PSUM partition (free row-broadcast)
            ones_mat = pp.tile([128, 128], BF16, tag="ones_mat")
            nc.vector.memset(ones_mat[:], 1.0)
            # PE pre-warm: ~4.5us of dummy matmuls while the first xT/wqk
            # DMAs are in flight, so the HAM clock gate reaches 8/8 before
            # stage B's first real matmul (cold MMs run at 1.2 GHz)
            warm_ps = psp.tile([128, 128], F32, tag="ps", name="warm_ps")
            for _ in range(40):
                nc.tensor.matmul(warm_ps[:], ones_mat[:],
                                 ones_mat[:], start=True, stop=True)
            # cos/sin ride the scalar-engine DMA queue so the sync queue's
            # head of line is the stage-B critical path (xT[0] + wqk[0])
            cosT = pp.tile([64, T], BF16, tag="cosT")
            nc.scalar.dma_start(cosT[:], cos_in)
            sinT = pp.tile([64, T], BF16, tag="sinT")
            nc.scalar.dma_start(sinT[:], sin_in)
            # causal 0/1 masks, variant m: keep (1) iff q_rel - k_rel - 128*m
            # >= 0.  Applied as a DVE multiply: the gpsimd queue must stay
            # clear for the AllGather's DMA burst (it head-of-line blocks).
            masks = []
            for m in range(4):
                mk = pp.tile([128, 512], BF16, tag=f"mask{m}")
                nc.gpsimd.memset(mk[:], 1.0)
                nc.gpsimd.affine_select(
                    out=mk[:], in_=mk[:], compare_op=mybir.AluOpType.is_ge,
                    fill=0.0, base=-128 * m, pattern=[[1, 512]],
                    channel_multiplier=-1)
                masks.append(mk)

            xT = [pp.tile([128, T], BF16, tag=f"xT{c}", name=f"xT{c}")
                  for c in range(CC)]
            # v_out projection weights are made resident early so the final
            # (collective-dependent) stage never waits on a weight stream
            wprT = [pp.tile([128, COLS], BF16, tag=f"wprT{c}", name=f"wprT{c}")
                    for c in range(CC)]

            with tc.tile_pool(name="vtpool", bufs=1) as vtp:
                vT = [vtp.tile([128, T], BF16, tag=f"vT{c}", name=f"vT{c}")
                      for c in range(CC)]

                with tc.tile_pool(name="qkpool", bufs=1) as qkp:
                    qT = [qkp.tile([128, T], BF16, tag=f"qT{j}", name=f"qT{j}")
                          for j in range(HL)]
                    kT = [qkp.tile([128, T], BF16, tag=f"kT{j}", name=f"kT{j}")
                          for j in range(HL)]
                    vp = [qkp.tile([128, COLS], BF16, tag=f"vp{t}", name=f"vp{t}")
                          for t in range(TT)]

                    # ---- stage B: q/k projection (transposed out) + RoPE ----
                    # 8 groups of 2 d-tiles -> 4 live PSUM accumulators/group.
                    # xT loads are interleaved with group 0's weight stream so
                    # TensorE starts ~immediately.  RoPE reads the PSUM halves
                    # directly (DVE TensorTensor with one PSUM operand).
                    for G in range(8):
                        _mark(nc, f'B{G}')
                        # odd groups borrow the (B-phase-idle) pyz ring for
                        # their dl=1 accumulators so consecutive groups are
                        # fully double-buffered with ps=6 + pyz=2 banks
                        psums = [[psp.tile([128, 512], F32,
                                           tag=("pyz" if (G % 2 and i) else "ps"),
                                           bufs=(2 if (G % 2 and i) else None),
                                           name=f"psB{G}_{i}_{q}")
                                  for q in range(QC)] for i in range(2)]
                        for c in range(CC):
                            if G == 0:
                                # xT streams on the gpsimd DMA engine so the
                                # sync engine's wqk stream never falls behind
                                nc.gpsimd.dma_start(xT[c][:], x_in[c])
                            wt = wsp.tile([128, 256], BF16, tag="wqk_s",
                                          bufs=8)
                            nc.sync.dma_start(wt[:], wqk_in[G, c])
                            for dl in range(2):
                                for qc in range(QC):
                                    nc.tensor.matmul(
                                        psums[dl][qc][:],
                                        wt[:, dl * 128:(dl + 1) * 128],
                                        xT[c][:, qc * 512:(qc + 1) * 512],
                                        start=(c == 0), stop=(c == CC - 1))
                        for dl in range(2):
                            dt = 2 * G + dl
                            dst = qT[dt] if dt < HL else kT[dt - HL]
                            for qc in range(QC):
                                ps = psums[dl][qc]
                                cs = slice(qc * 512, (qc + 1) * 512)
                                # 4 DVE mults read the PSUM halves directly
                                # (frees the bank after the 4th); the two
                                # combines run on the otherwise-idle GpSimd
                                t1 = rtp.tile([64, 512], BF16, tag="rt1")
                                nc.vector.tensor_tensor(
                                    t1[:], ps[0:64, :], cosT[:, cs],
                                    mybir.AluOpType.mult)
                                t2 = rtp.tile([64, 512], BF16, tag="rt2")
                                nc.vector.tensor_tensor(
                                    t2[:], ps[64:128, :], sinT[:, cs],
                                    mybir.AluOpType.mult)
                                t3 = rtp.tile([64, 512], BF16, tag="rt3")
                                nc.vector.tensor_tensor(
                                    t3[:], ps[0:64, :], sinT[:, cs],
                                    mybir.AluOpType.mult)
                                t4 = rtp.tile([64, 512], BF16, tag="rt4")
                                nc.vector.tensor_tensor(
                                    t4[:], ps[64:128, :], cosT[:, cs],
                                    mybir.AluOpType.mult)
                                nc.gpsimd.tensor_tensor(
                                    dst[0:64, cs], t1[:], t2[:],
                                    mybir.AluOpType.subtract)
                                nc.gpsimd.tensor_tensor(
                                    dst[64:128, cs], t3[:], t4[:],
                                    mybir.AluOpType.add)

                    # vT arrives while stage B computes; wprT is triggered
                    # after C1 (landing during D1) to keep the B/C0 windows'
                    # HBM bandwidth for the wqk/wat/xT/vT streams
                    for c in range(CC):
                        nc.sync.dma_start(vT[c][:], v_in[c])

                    def vp_group(tg, tlist, copy_engine, pyz2=False):
                        # vp[t] for t in tlist: 16-chunk contraction.  pyz2
                        # puts the 2nd tile's accumulators in the pyz ring so
                        # back-to-back subgroups double-buffer within 6+2 banks
                        _mark(nc, f'C{tg}')
                        last = len(tlist) - 1
                        psums = [[psp.tile([128, 512], F32,
                                           tag=("pyz" if (pyz2 and i == last)
                                                else "ps"),
                                           bufs=(2 if (pyz2 and i == last)
                                                 else None),
                                           name=f"psC{t}_{q}")
                                  for q in range(2)] for i, t in enumerate(tlist)]
                        for c in range(CC):
                            wt = wsp.tile([128, COLS], BF16, tag="wat_s",
                                          bufs=6)
                            nc.sync.dma_start(wt[:], wat_in[c])
                            for i, t in enumerate(tlist):
                                for dh in range(2):
                                    nc.tensor.matmul(
                                        psums[i][dh][:],
                                        vT[c][:, t * 128:(t + 1) * 128],
                                        wt[:, dh * 512:(dh + 1) * 512],
                                        start=(c == 0), stop=(c == CC - 1))
                        for i, t in enumerate(tlist):
                            for dh in range(2):
                                eng = copy_engine[(i * 2 + dh) % len(copy_engine)]
                                if eng == 's':
                                    nc.scalar.copy(
                                        vp[t][:, dh * 512:(dh + 1) * 512],
                                        psums[i][dh][:])
                                else:
                                    nc.vector.tensor_copy(
                                        vp[t][:, dh * 512:(dh + 1) * 512],
                                        psums[i][dh][:])

                    def attn_head(j, qc):
                        # one head's attention for token half qc.  py/pz live
                        # in their own 2-bank ring so the next head's score
                        # tiles never wait on this head's normalization.
                        ktiles = 4 * qc + 4
                        py = psp.tile([128, 512], F32, tag="pyz", bufs=2,
                                      name=f"py{j}_{qc}")
                        pz = psp.tile([128, 512], F32, tag="pyz", bufs=2,
                                      name=f"pz{j}_{qc}")
                        for ki in range(ktiles):
                            pscr = psp.tile([128, 512], F32, tag="ps",
                                            name=f"pscr{j}_{qc}_{ki}")
                            nc.tensor.matmul(
                                pscr[:],
                                kT[j][:, ki * 128:(ki + 1) * 128],
                                qT[j][:, qc * 512:(qc + 1) * 512],
                                start=True, stop=True)
                            pt = ptp.tile([128, 512], BF16, tag="p_t")
                            nc.scalar.activation(
                                pt[:], pscr[:],
                                mybir.ActivationFunctionType.Exp)
                            m = ki - 4 * qc
                            if m >= 0:
                                nc.vector.tensor_tensor(
                                    pt[:], pt[:], masks[m][:],
                                    mybir.AluOpType.mult)
                            nc.tensor.matmul(
                                py[:], vp[ki][:, j * 128:(j + 1) * 128], pt[:],
                                start=(ki == 0), stop=(ki == ktiles - 1))
                            nc.tensor.matmul(
                                pz[:], ones_mat[:], pt[:],
                                start=(ki == 0), stop=(ki == ktiles - 1))
                        zr = rtp.tile([128, 512], F32, tag="zrec")
                        nc.vector.reciprocal_approx_fast(zr[:], pz[:])
                        yt = ptp.tile([128, 512], BF16, tag="y_t")
                        nc.vector.tensor_tensor(
                            yt[:], py[:], zr[:], mybir.AluOpType.mult)
                        nc.sync.dma_start(y_bounce[qc].ap()[j], yt[:])

                    def proj(w_of, lhs_fn, dst, tlist, psname, copy_engine,
                             pyz2=False):
                        # out[t, :] += lhs[c]^T @ w[c] over 16 chunks
                        _mark(nc, f'P{psname}')
                        last = len(tlist) - 1
                        psums = [[psp.tile([128, 512], F32,
                                           tag=("pyz" if (pyz2 and i == last)
                                                else "ps"),
                                           bufs=(2 if (pyz2 and i == last)
                                                 else None),
                                           name=f"ps{psname}_{t}_{q}")
                                  for q in range(2)] for i, t in enumerate(tlist)]
                        for c in range(CC):
                            wt = w_of(c)
                            for i, t in enumerate(tlist):
                                for dh in range(2):
                                    nc.tensor.matmul(
                                        psums[i][dh][:],
                                        lhs_fn(c, t),
                                        wt[:, dh * 512:(dh + 1) * 512],
                                        start=(c == 0), stop=(c == CC - 1))
                        for i, t in enumerate(tlist):
                            ot = ocp.tile([128, COLS], BF16, tag="out_cp")
                            for dh in range(2):
                                eng = copy_engine[(i * 2 + dh) % len(copy_engine)]
                                if eng == 's':
                                    nc.scalar.copy(
                                        ot[:, dh * 512:(dh + 1) * 512],
                                        psums[i][dh][:])
                                else:
                                    nc.vector.tensor_copy(
                                        ot[:, dh * 512:(dh + 1) * 512],
                                        psums[i][dh][:])
                            nc.sync.dma_start(dst[t], ot[:])

                    def wep_stream(c):
                        wt = wsp.tile([128, COLS], BF16, tag="wo_s")
                        nc.sync.dma_start(wt[:], wep_in[c])
                        return wt

                    xT_lhs = lambda c, t: xT[c][:, t * 128:(t + 1) * 128]

                    # ---- C0: vp tiles 0-3 (scalar evacuates; no exp nearby).
                    # Full 4-tile group: tiles 0-2 in the ps ring, tile 3
                    # borrows pyz, so W_attn streams once per half ----
                    vp_group(0, [0, 1, 2, 3], ['s', 'v'], pyz2=True)

                    # preload the Exp activation table off D0's critical path
                    # (the first Exp after C0's Copy activations reloads it)
                    dume = ptp.tile([1, 16], BF16, tag="dume")
                    nc.scalar.activation(dume[:], ones_mat[0:1, 0:16],
                                         mybir.ActivationFunctionType.Exp)

                    # ---- D0 (attention half 0) ----
                    _mark(nc, 'D0')
                    for j in range(HL):
                        attn_head(j, 0)
                    nc.gpsimd.collective_compute(
                        "AllGather", mybir.AluOpType.bypass,
                        replica_groups=[[0, 1], [2, 3], [4, 5], [6, 7]],
                        ins=[y_bounce[0].ap()], outs=[y_gather[0].ap()])

                    # ---- C1: vp tiles 4-7 (DVE evacuates; ScalarE owns Exp) --
                    vp_group(1, [4, 5, 6, 7], ['v'], pyz2=True)
                    # wprT triggers retire on the scalar queue during C1
                    # (no exps there), landing while D1 computes
                    for c in range(CC):
                        nc.scalar.dma_start(wprT[c][:], wpr_in[c])

                    # ---- D1 (attention half 1) with x_eps_out interleaved so
                    # TensorE has slack work while ScalarE streams exps ----
                    _mark(nc, 'D1')
                    attn_head(0, 1)
                    attn_head(1, 1)
                    proj(wep_stream, xT_lhs, x_out, [0, 1], "X0a", ['v'])
                    attn_head(2, 1)
                    attn_head(3, 1)
                    proj(wep_stream, xT_lhs, x_out, [2, 3], "X0b", ['v'])
                    attn_head(4, 1)
                    attn_head(5, 1)
                    proj(wep_stream, xT_lhs, x_out, [4, 5], "X1a", ['v'])
                    attn_head(6, 1)
                    attn_head(7, 1)
                    nc.gpsimd.collective_compute(
                        "AllGather", mybir.AluOpType.bypass,
                        replica_groups=[[0, 1], [2, 3], [4, 5], [6, 7]],
                        ins=[y_bounce[1].ap()], outs=[y_gather[1].ap()])

            # ---- tail: v_out projections + remaining x_eps_out ----
            # v_out rows [qc*512, ..+512) consume y columns of the same token
            # range, so group qc only needs exchange chunk qc.
            with tc.tile_pool(name="ygpool", bufs=1) as ygp, \
                 tc.tile_pool(name="wepx", bufs=1) as wxp:
                yG = [[ygp.tile([128, 512], BF16, tag=f"yG{qc}_{c}",
                                name=f"yG{qc}_{c}") for c in range(CC)]
                      for qc in range(QC)]
                # X1b's W_eps_proj chunks become resident in the SBUF space
                # vacated by vT/qT/kT/vp, streamed during V0 (no tail stall)
                wepx = [wxp.tile([128, COLS], BF16, tag=f"wepx{c}",
                                 name=f"wepx{c}") for c in range(CC)]
                for c in range(CC):
                    nc.sync.dma_start(wepx[c][:], wep_in[c])
                yG_lhs = lambda c, t: yG[t // 4][c][:, (t % 4) * 128:
                                                   (t % 4) * 128 + 128]
                # both gather halves prefetch on the gpsimd queue as soon as
                # their AllGather lands (yG[0] during late D1, yG[1] during V0)
                # both gather halves prefetch on the gpsimd queue as soon as
                # their AllGather lands (yG[0] during late D1, yG[1] during V0)
                for c in range(CC):
                    nc.gpsimd.dma_start(yG[0][c][:], y_gather[0].ap()[c])
                for c in range(CC):
                    nc.gpsimd.dma_start(yG[1][c][:], y_gather[1].ap()[c])
                proj(lambda c: wprT[c], yG_lhs, v_out, [0, 1], "V0a",
                     ['s', 'v'])
                proj(lambda c: wprT[c], yG_lhs, v_out, [2, 3], "V0b",
                     ['s', 'v'], pyz2=True)
                proj(lambda c: wepx[c], xT_lhs, x_out, [6, 7], "X1b",
                     ['s', 'v'])
                proj(lambda c: wprT[c], yG_lhs, v_out, [4, 5], "V1a",
                     ['s', 'v'], pyz2=True)
                proj(lambda c: wprT[c], yG_lhs, v_out, [6, 7], "V1b",
                     ['s', 'v'])

    nc.compile()
    return nc


def _prep_core_inputs(inputs):
    """Host-side shard prep: slicing, bf16 cast, weight permutation/packing."""
    x_eps = np.asarray(inputs["x_eps"], np.float32)
    v = np.asarray(inputs["v"], np.float32)
    cos = np.asarray(inputs["freqs_cos"], np.float32)
    sin = np.asarray(inputs["freqs_sin"], np.float32)
    Wqk = np.asarray(inputs["W_eps_attn"], np.float32)
    Wat = np.asarray(inputs["W_attn"], np.float32)
    Wpr = np.asarray(inputs["W_proj"], np.float32)
    Wep = np.asarray(inputs["W_eps_proj"], np.float32)

    cosT = np.ascontiguousarray(cos.T).astype(BF16_NP)     # [64, T]
    sinT = np.ascontiguousarray(sin.T).astype(BF16_NP)
    perm = np.concatenate([np.arange(0, HD, 2), np.arange(1, HD, 2)])  # rotate-half
    scale = np.float32(1.0 / np.sqrt(HD))

    # contraction-major (transposed) bf16 activations, tiled [CC, 128, T]
    xT_bf = [np.ascontiguousarray(x_eps[b].astype(BF16_NP).T)
             .reshape(CC, 128, T) for b in range(B)]
    vT_bf = [np.ascontiguousarray(v[b].astype(BF16_NP).T)
             .reshape(CC, 128, T) for b in range(B)]

    per_g = []
    for g in range(2):
        heads = range(g * HL, (g + 1) * HL)
        wq = np.concatenate(
            [Wqk[:, h * HD:(h + 1) * HD][:, perm] * scale for h in heads], axis=1)
        wk = np.concatenate(
            [Wqk[:, DIM + h * HD:DIM + (h + 1) * HD][:, perm] for h in heads],
            axis=1)
        wqk_cols = np.concatenate([wq, wk], axis=1)          # [2048, 2048]
        wqk_packed = np.empty((8, CC, 128, 256), np.float32)
        for G in range(8):
            for dl in range(2):
                dt = 2 * G + dl
                blk = wqk_cols[:, dt * 128:(dt + 1) * 128]    # [2048, 128]
                wqk_packed[G, :, :, dl * 128:(dl + 1) * 128] = \
                    blk.reshape(CC, 128, 128)
        cols = slice(g * COLS, (g + 1) * COLS)
        per_g.append({
            "w_qk": wqk_packed.astype(BF16_NP),
            "w_attn": Wat[:, cols].reshape(CC, 128, COLS).astype(BF16_NP),
            "w_proj": Wpr[:, cols].reshape(CC, 128, COLS).astype(BF16_NP),
            "w_eps_proj": Wep[:, cols].reshape(CC, 128, COLS).astype(BF16_NP),
        })

    in_maps = []
    for core in range(NCORES):
        b, g = divmod(core, 2)
        in_maps.append({
            "xT_bf": xT_bf[b],
            "vT_bf": vT_bf[b],
            "cos_t": cosT,
            "sin_t": sinT,
            **per_g[g],
        })
    return in_maps


def _get_compiled():
    global _COMPILED
    if _COMPILED is None:
        _COMPILED = _build()
    return _COMPILED


def kernel(**inputs):
    nc = _get_compiled()
    in_maps = _prep_core_inputs(inputs)
    res = run_bass_kernel_spmd(nc, in_maps, list(range(NCORES)))
    v_out = np.empty((B, T, DIM), np.float32)
    x_out = np.empty((B, T, DIM), np.float32)
    for core in range(NCORES):
        b, g = divmod(core, 2)
        r = res.results[core]
        cols = slice(g * COLS, (g + 1) * COLS)
        v_out[b][:, cols] = r["v_out"].reshape(T, COLS).astype(np.float32)
        x_out[b][:, cols] = r["x_out"].reshape(T, COLS).astype(np.float32)
    return (v_out, x_out)


# revision 55
# speedup vs baseline: 1.0104x; 1.0104x over previous
"""Distributed Trainium2 (Bass/Tile) kernel for a causal self-attention block.

Reference computation (per batch b):
    qk = x_eps @ W_eps_attn ; q,k = split(qk) ; vp = v @ W_attn
    q,k = rope(q), rope(k)   (llama-style, 16 heads x 128 dims)
    y   = causal_softmax(q k^T / sqrt(128)) @ vp   (per head)
    v_out     = y @ W_proj
    x_eps_out = x_eps @ W_eps_proj

Sharding over 8 NeuronCores: core = (b, g) = 4-way batch x 2-way head-group
(8 heads per core).  W_eps_attn/W_attn are column-sharded by head; y is
exchanged pair-wise in two chunks (AllGather per 512-token half, pipelined
behind later compute) and W_proj/W_eps_proj are used column-sharded so each
core produces a disjoint half of both outputs (no reduce needed).

All matmuls run in bf16 with fp32 PSUM accumulation.  Activations are
uploaded contraction-major (pre-transposed host-side, numerically identical
to a device-side transpose of the same bf16 data); the rotate-half RoPE
layout, the 1/sqrt(128) q-scale and all weight tiling/permutation are pure
host-side weight re-layout.

Engine balance (v2): RoPE multiplies read the PSUM directly (no DVE
pre-copies), the causal mask is applied in-place on GpSimd via
affine_select, the softmax denominator uses the fast approximate DVE
reciprocal, vp/out PSUM evacuation is split between ScalarE and DVE, and
the x_eps_out projection is interleaved into the second attention half so
TensorE never waits on ScalarE's exp stream.
"""

import sys

sys.path.insert(0, "/opt/trn_rl_repo")

import numpy as np
import ml_dtypes

import concourse.bass as bass
import concourse.mybir as mybir
import concourse.tile as tile
from concourse import bacc
from concourse.bass_utils import run_bass_kernel_spmd

F32 = mybir.dt.float32
BF16 = mybir.dt.bfloat16
BF16_NP = ml_dtypes.bfloat16

B, T, DIM, H, HD = 4, 1024, 2048, 16, 128
NCORES = 8
HL = H // 2          # heads per core (8)
TT = T // 128        # t-tiles (8)
CC = DIM // 128      # contraction chunks (16)
QC = T // 512        # 512-wide q chunks (2)
COLS = DIM // 2      # local column count of each output half (1024)

_COMPILED = None
STAGE_MARKS = []


def _mark(nc, name):
    STAGE_MARKS.append((name, len(nc.inst_map)))


def _build():
    nc = bacc.Bacc(trn_type="TRN2", target_bir_lowering=False, debug=False,
                   num_devices=NCORES)

    # ---- per-core I/O (activations contraction-major, weights pre-packed) ----
    x_in = nc.dram_tensor("xT_bf", [CC, 128, T], BF16, kind="ExternalInput").ap()
    v_in = nc.dram_tensor("vT_bf", [CC, 128, T], BF16, kind="ExternalInput").ap()
    cos_in = nc.dram_tensor("cos_t", [64, T], BF16, kind="ExternalInput").ap()
    sin_in = nc.dram_tensor("sin_t", [64, T], BF16, kind="ExternalInput").ap()
    wqk_in = nc.dram_tensor("w_qk", [8, CC, 128, 256], BF16, kind="ExternalInput").ap()
    wat_in = nc.dram_tensor("w_attn", [CC, 128, COLS], BF16, kind="ExternalInput").ap()
    wpr_in = nc.dram_tensor("w_proj", [CC, 128, COLS], BF16, kind="ExternalInput").ap()
    wep_in = nc.dram_tensor("w_eps_proj", [CC, 128, COLS], BF16,
                            kind="ExternalInput").ap()
    v_out = nc.dram_tensor("v_out", [TT, 128, COLS], BF16, kind="ExternalOutput").ap()
    x_out = nc.dram_tensor("x_out", [TT, 128, COLS], BF16, kind="ExternalOutput").ap()

    # internal DRAM for the chunked pair-wise y exchange
    y_bounce = [nc.dram_tensor(f"y_bounce{qc}", [HL, 128, 512], BF16)
                for qc in range(QC)]
    y_gather = [nc.dram_tensor(f"y_gather{qc}", [2 * HL, 128, 512], BF16)
                for qc in range(QC)]

    with tile.TileContext(nc) as tc:
        with tc.tile_pool(name="persist", bufs=1) as pp, \
             tc.tile_pool(name="wstream", bufs=4) as wsp, \
             tc.tile_pool(name="ptile", bufs=6) as ptp, \
             tc.tile_pool(name="ropetmp", bufs=2) as rtp, \
             tc.tile_pool(name="outcp", bufs=3) as ocp, \
             tc.tile_pool(name="ps", bufs=6, space="PSUM") as psp:
            # PSUM budget: tag "ps" 6 banks (B groups, vp/proj subgroups,
            # score scratch) + tag "pyz" 2 banks (attention accumulators)

            # all-ones stationary operand: the denominator matmul then lands
            # Z replicated on every PSUM partition (free row-broadcast)
            ones_mat = pp.tile([128, 128], BF16, tag="ones_mat")
            nc.vector.memset(ones_mat[:], 1.0)
            # PE pre-warm: ~4.5us of dummy matmuls while the first xT/wqk
            # DMAs are in flight, so the HAM clock gate reaches 8/8 before
            # stage B's first real matmul (cold MMs run at 1.2 GHz)
            warm_ps = psp.tile([128, 128], F32, tag="ps", name="warm_ps")
            for _ in range(40):
                nc.tensor.matmul(warm_ps[:], ones_mat[:],
                                 ones_mat[:], start=True, stop=True)
            # cos/sin ride the scalar-engine DMA queue so the sync queue's
            # head of line is the stage-B critical path (xT[0] + wqk[0])
            cosT = pp.tile([64, T], BF16, tag="cosT")
            nc.scalar.dma_start(cosT[:], cos_in)
            sinT = pp.tile([64, T], BF16, tag="sinT")
            nc.scalar.dma_start(sinT[:], sin_in)
            # causal 0/1 masks, variant m: keep (1) iff q_rel - k_rel - 128*m
            # >= 0.  Applied as a DVE multiply: the gpsimd queue must stay
            # clear for the AllGather's DMA burst (it head-of-line blocks).
            masks = []
            for m in range(4):
                mk = pp.tile([128, 512], BF16, tag=f"mask{m}")
                nc.gpsimd.memset(mk[:], 1.0)
                nc.gpsimd.affine_select(
                    out=mk[:], in_=mk[:], compare_op=mybir.AluOpType.is_ge,
                    fill=0.0, base=-128 * m, pattern=[[1, 512]],
                    channel_multiplier=-1)
                masks.append(mk)

            xT = [pp.tile([128, T], BF16, tag=f"xT{c}", name=f"xT{c}")
                  for c in range(CC)]
            # v_out projection weights are made resident early so the final
            # (collective-dependent) stage never waits on a weight stream
            wprT = [pp.tile([128, COLS], BF16, tag=f"wprT{c}", name=f"wprT{c}")
                    for c in range(CC)]

            with tc.tile_pool(name="vtpool", bufs=1) as vtp:
                vT = [vtp.tile([128, T], BF16, tag=f"vT{c}", name=f"vT{c}")
                      for c in range(CC)]

                with tc.tile_pool(name="qkpool", bufs=1) as qkp:
                    qT = [qkp.tile([128, T], BF16, tag=f"qT{j}", name=f"qT{j}")
                          for j in range(HL)]
                    kT = [qkp.tile([128, T], BF16, tag=f"kT{j}", name=f"kT{j}")
                          for j in range(HL)]
                    vp = [qkp.tile([128, COLS], BF16, tag=f"vp{t}", name=f"vp{t}")
                          for t in range(TT)]

                    # ---- stage B: q/k projection (transposed out) + RoPE ----
                    # 8 groups of 2 d-tiles -> 4 live PSUM accumulators/group.
                    # xT loads are interleaved with group 0's weight stream so
                    # TensorE starts ~immediately.  RoPE reads the PSUM halves
                    # directly (DVE TensorTensor with one PSUM operand).
                    for G in range(8):
                        _mark(nc, f'B{G}')
                        # odd groups borrow the (B-phase-idle) pyz ring for
                        # their dl=1 accumulators so consecutive groups are
                        # fully double-buffered with ps=6 + pyz=2 banks
                        psums = [[psp.tile([128, 512], F32,
                                           tag=("pyz" if (G % 2 and i) else "ps"),
                                           bufs=(2 if (G % 2 and i) else None),
                                           name=f"psB{G}_{i}_{q}")
                                  for q in range(QC)] for i in range(2)]
                        for c in range(CC):
                            if G == 0:
                                nc.sync.dma_start(xT[c][:], x_in[c])
                            wt = wsp.tile([128, 256], BF16, tag="wqk_s",
                                          bufs=8)
                            nc.sync.dma_start(wt[:], wqk_in[G, c])
                            for dl in range(2):
                                for qc in range(QC):
                                    nc.tensor.matmul(
                                        psums[dl][qc][:],
                                        wt[:, dl * 128:(dl + 1) * 128],
                                        xT[c][:, qc * 512:(qc + 1) * 512],
                                        start=(c == 0), stop=(c == CC - 1))
                        for dl in range(2):
                            dt = 2 * G + dl
                            dst = qT[dt] if dt < HL else kT[dt - HL]
                            for qc in range(QC):
                                ps = psums[dl][qc]
                                cs = slice(qc * 512, (qc + 1) * 512)
                                # 4 DVE mults read the PSUM halves directly
                                # (frees the bank after the 4th); the two
                                # combines run on the otherwise-idle GpSimd
                                t1 = rtp.tile([64, 512], BF16, tag="rt1")
                                nc.vector.tensor_tensor(
                                    t1[:], ps[0:64, :], cosT[:, cs],
                                    mybir.AluOpType.mult)
                                t2 = rtp.tile([64, 512], BF16, tag="rt2")
                                nc.vector.tensor_tensor(
                                    t2[:], ps[64:128, :], sinT[:, cs],
                                    mybir.AluOpType.mult)
                                t3 = rtp.tile([64, 512], BF16, tag="rt3")
                                nc.vector.tensor_tensor(
                                    t3[:], ps[0:64, :], sinT[:, cs],
                                    mybir.AluOpType.mult)
                                t4 = rtp.tile([64, 512], BF16, tag="rt4")
                                nc.vector.tensor_tensor(
                                    t4[:], ps[64:128, :], cosT[:, cs],
                                    mybir.AluOpType.mult)
                                nc.gpsimd.tensor_tensor(
                                    dst[0:64, cs], t1[:], t2[:],
                                    mybir.AluOpType.subtract)
                                nc.gpsimd.tensor_tensor(
                                    dst[64:128, cs], t3[:], t4[:],
                                    mybir.AluOpType.add)

                    # vT arrives while stage B computes
                    for c in range(CC):
                        nc.sync.dma_start(vT[c][:], v_in[c])
                    for c in range(CC):
                        nc.sync.dma_start(wprT[c][:], wpr_in[c])

                    def vp_group(tg, tlist, copy_engine, pyz2=False):
                        # vp[t] for t in tlist: 16-chunk contraction.  pyz2
                        # puts the 2nd tile's accumulators in the pyz ring so
                        # back-to-back subgroups double-buffer within 6+2 banks
                        _mark(nc, f'C{tg}')
                        last = len(tlist) - 1
                        psums = [[psp.tile([128, 512], F32,
                                           tag=("pyz" if (pyz2 and i == last)
                                                else "ps"),
                                           bufs=(2 if (pyz2 and i == last)
                                                 else None),
                                           name=f"psC{t}_{q}")
                                  for q in range(2)] for i, t in enumerate(tlist)]
                        for c in range(CC):
                            wt = wsp.tile([128, COLS], BF16, tag="wat_s",
                                          bufs=6)
                            nc.sync.dma_start(wt[:], wat_in[c])
                            for i, t in enumerate(tlist):
                                for dh in range(2):
                                    nc.tensor.matmul(
                                        psums[i][dh][:],
                                        vT[c][:, t * 128:(t + 1) * 128],
                                        wt[:, dh * 512:(dh + 1) * 512],
                                        start=(c == 0), stop=(c == CC - 1))
                        for i, t in enumerate(tlist):
                            for dh in range(2):
                                eng = copy_engine[(i * 2 + dh) % len(copy_engine)]
                                if eng == 's':
                                    nc.scalar.copy(
                                        vp[t][:, dh * 512:(dh + 1) * 512],
                                        psums[i][dh][:])
                                else:
                                    nc.vector.tensor_copy(
                                        vp[t][:, dh * 512:(dh + 1) * 512],
                                        psums[i][dh][:])

                    def attn_head(j, qc):
                        # one head's attention for token half qc.  py/pz live
                        # in their own 2-bank ring so the next head's score
                        # tiles never wait on this head's normalization.
                        ktiles = 4 * qc + 4
                        py = psp.tile([128, 512], F32, tag="pyz", bufs=2,
                                      name=f"py{j}_{qc}")
                        pz = psp.tile([128, 512], F32, tag="pyz", bufs=2,
                                      name=f"pz{j}_{qc}")
                        for ki in range(ktiles):
                            pscr = psp.tile([128, 512], F32, tag="ps",
                                            name=f"pscr{j}_{qc}_{ki}")
                            nc.tensor.matmul(
                                pscr[:],
                                kT[j][:, ki * 128:(ki + 1) * 128],
                                qT[j][:, qc * 512:(qc + 1) * 512],
                                start=True, stop=True)
                            pt = ptp.tile([128, 512], BF16, tag="p_t")
                            nc.scalar.activation(
                                pt[:], pscr[:],
                                mybir.ActivationFunctionType.Exp)
                            m = ki - 4 * qc
                            if m >= 0:
                                nc.vector.tensor_tensor(
                                    pt[:], pt[:], masks[m][:],
                                    mybir.AluOpType.mult)
                            nc.tensor.matmul(
                                py[:], vp[ki][:, j * 128:(j + 1) * 128], pt[:],
                                start=(ki == 0), stop=(ki == ktiles - 1))
                            nc.tensor.matmul(
                                pz[:], ones_mat[:], pt[:],
                                start=(ki == 0), stop=(ki == ktiles - 1))
                        zr = rtp.tile([128, 512], F32, tag="zrec")
                        nc.vector.reciprocal_approx_fast(zr[:], pz[:])
                        yt = ptp.tile([128, 512], BF16, tag="y_t")
                        nc.vector.tensor_tensor(
                            yt[:], py[:], zr[:], mybir.AluOpType.mult)
                        nc.sync.dma_start(y_bounce[qc].ap()[j], yt[:])

                    def proj(w_of, lhs_fn, dst, tlist, psname, copy_engine,
                             pyz2=False):
                        # out[t, :] += lhs[c]^T @ w[c] over 16 chunks
                        _mark(nc, f'P{psname}')
                        last = len(tlist) - 1
                        psums = [[psp.tile([128, 512], F32,
                                           tag=("pyz" if (pyz2 and i == last)
                                                else "ps"),
                                           bufs=(2 if (pyz2 and i == last)
                                                 else None),
                                           name=f"ps{psname}_{t}_{q}")
                                  for q in range(2)] for i, t in enumerate(tlist)]
                        for c in range(CC):
                            wt = w_of(c)
                            for i, t in enumerate(tlist):
                                for dh in range(2):
                                    nc.tensor.matmul(
                                        psums[i][dh][:],
                                        lhs_fn(c, t),
                                        wt[:, dh * 512:(dh + 1) * 512],
                                        start=(c == 0), stop=(c == CC - 1))
                        for i, t in enumerate(tlist):
                            ot = ocp.tile([128, COLS], BF16, tag="out_cp")
                            for dh in range(2):
                                eng = copy_engine[(i * 2 + dh) % len(copy_engine)]
                                if eng == 's':
                                    nc.scalar.copy(
                                        ot[:, dh * 512:(dh + 1) * 512],
                                        psums[i][dh][:])
                                else:
                                    nc.vector.tensor_copy(
                                        ot[:, dh * 512:(dh + 1) * 512],
                                        psums[i][dh][:])
                            nc.sync.dma_start(dst[t], ot[:])

                    def wep_stream(c):
                        wt = wsp.tile([128, COLS], BF16, tag="wo_s")
                        nc.sync.dma_start(wt[:], wep_in[c])
                        return wt

                    xT_lhs = lambda c, t: xT[c][:, t * 128:(t + 1) * 128]

                    # ---- C0: vp tiles 0-3 (scalar evacuates; no exp nearby).
                    # Full 4-tile group: tiles 0-2 in the ps ring, tile 3
                    # borrows pyz, so W_attn streams once per half ----
                    vp_group(0, [0, 1, 2, 3], ['s', 'v'], pyz2=True)

                    # preload the Exp activation table off D0's critical path
                    # (the first Exp after C0's Copy activations reloads it)
                    dume = ptp.tile([1, 16], BF16, tag="dume")
                    nc.scalar.activation(dume[:], ones_mat[0:1, 0:16],
                                         mybir.ActivationFunctionType.Exp)

                    # ---- D0 (attention half 0) ----
                    _mark(nc, 'D0')
                    for j in range(HL):
                        attn_head(j, 0)
                    nc.gpsimd.collective_compute(
                        "AllGather", mybir.AluOpType.bypass,
                        replica_groups=[[0, 1], [2, 3], [4, 5], [6, 7]],
                        ins=[y_bounce[0].ap()], outs=[y_gather[0].ap()])

                    # ---- C1: vp tiles 4-7 (DVE evacuates; ScalarE owns Exp) --
                    vp_group(1, [4, 5, 6, 7], ['v'], pyz2=True)

                    # ---- D1 (attention half 1) with x_eps_out interleaved so
                    # TensorE has slack work while ScalarE streams exps ----
                    _mark(nc, 'D1')
                    attn_head(0, 1)
                    attn_head(1, 1)
                    proj(wep_stream, xT_lhs, x_out, [0, 1], "X0a", ['v'])
                    attn_head(2, 1)
                    attn_head(3, 1)
                    proj(wep_stream, xT_lhs, x_out, [2, 3], "X0b", ['v'])
                    attn_head(4, 1)
                    attn_head(5, 1)
                    proj(wep_stream, xT_lhs, x_out, [4, 5], "X1a", ['v'])
                    attn_head(6, 1)
                    attn_head(7, 1)
                    nc.gpsimd.collective_compute(
                        "AllGather", mybir.AluOpType.bypass,
                        replica_groups=[[0, 1], [2, 3], [4, 5], [6, 7]],
                        ins=[y_bounce[1].ap()], outs=[y_gather[1].ap()])

            # ---- tail: v_out projections + remaining x_eps_out ----
            # v_out rows [qc*512, ..+512) consume y columns of the same token
            # range, so group qc only needs exchange chunk qc.
            with tc.tile_pool(name="ygpool", bufs=1) as ygp, \
                 tc.tile_pool(name="wepx", bufs=1) as wxp:
                yG = [[ygp.tile([128, 512], BF16, tag=f"yG{qc}_{c}",
                                name=f"yG{qc}_{c}") for c in range(CC)]
                      for qc in range(QC)]
                # X1b's W_eps_proj chunks become resident in the SBUF space
                # vacated by vT/qT/kT/vp, streamed during V0 (no tail stall)
                wepx = [wxp.tile([128, COLS], BF16, tag=f"wepx{c}",
                                 name=f"wepx{c}") for c in range(CC)]
                for c in range(CC):
                    nc.sync.dma_start(wepx[c][:], wep_in[c])
                yG_lhs = lambda c, t: yG[t // 4][c][:, (t % 4) * 128:
                                                   (t % 4) * 128 + 128]
                # both gather halves prefetch on the gpsimd queue as soon as
                # their AllGather lands (yG[0] during late D1, yG[1] during V0)
                # both gather halves prefetch on the gpsimd queue as soon as
                # their AllGather lands (yG[0] during late D1, yG[1] during V0)
                for c in range(CC):
                    nc.gpsimd.dma_start(yG[0][c][:], y_gather[0].ap()[c])
                for c in range(CC):
                    nc.gpsimd.dma_start(yG[1][c][:], y_gather[1].ap()[c])
                proj(lambda c: wprT[c], yG_lhs, v_out, [0, 1], "V0a",
                     ['s', 'v'])
                proj(lambda c: wprT[c], yG_lhs, v_out, [2, 3], "V0b",
                     ['s', 'v'], pyz2=True)
                proj(lambda c: wepx[c], xT_lhs, x_out, [6, 7], "X1b",
                     ['s', 'v'])
                proj(lambda c: wprT[c], yG_lhs, v_out, [4, 5], "V1a",
                     ['s', 'v'], pyz2=True)
                proj(lambda c: wprT[c], yG_lhs, v_out, [6, 7], "V1b",
                     ['s', 'v'])

    nc.compile()
    return nc


def _prep_core_inputs(inputs):
    """Host-side shard prep: slicing, bf16 cast, weight permutation/packing."""
    x_eps = np.asarray(inputs["x_eps"], np.float32)
    v = np.asarray(inputs["v"], np.float32)
    cos = np.asarray(inputs["freqs_cos"], np.float32)
    sin = np.asarray(inputs["freqs_sin"], np.float32)
    Wqk = np.asarray(inputs["W_eps_attn"], np.float32)
    Wat = np.asarray(inputs["W_attn"], np.float32)
    Wpr = np.asarray(inputs["W_proj"], np.float32)
    Wep = np.asarray(inputs["W_eps_proj"], np.float32)

    cosT = np.ascontiguousarray(cos.T).astype(BF16_NP)     # [64, T]
    sinT = np.ascontiguousarray(sin.T).astype(BF16_NP)
    perm = np.concatenate([np.arange(0, HD, 2), np.arange(1, HD, 2)])  # rotate-half
    scale = np.float32(1.0 / np.sqrt(HD))

    # contraction-major (transposed) bf16 activations, tiled [CC, 128, T]
    xT_bf = [np.ascontiguousarray(x_eps[b].astype(BF16_NP).T)
             .reshape(CC, 128, T) for b in range(B)]
    vT_bf = [np.ascontiguousarray(v[b].astype(BF16_NP).T)
             .reshape(CC, 128, T) for b in range(B)]

    per_g = []
    for g in range(2):
        heads = range(g * HL, (g + 1) * HL)
        wq = np.concatenate(
            [Wqk[:, h * HD:(h + 1) * HD][:, perm] * scale for h in heads], axis=1)
        wk = np.concatenate(
            [Wqk[:, DIM + h * HD:DIM + (h + 1) * HD][:, perm] for h in heads],
            axis=1)
        wqk_cols = np.concatenate([wq, wk], axis=1)          # [2048, 2048]
        wqk_packed = np.empty((8, CC, 128, 256), np.float32)
        for G in range(8):
            for dl in range(2):
                dt = 2 * G + dl
                blk = wqk_cols[:, dt * 128:(dt + 1) * 128]    # [2048, 128]
                wqk_packed[G, :, :, dl * 128:(dl + 1) * 128] = \
                    blk.reshape(CC, 128, 128)
        cols = slice(g * COLS, (g + 1) * COLS)
        per_g.append({
            "w_qk": wqk_packed.astype(BF16_NP),
            "w_attn": Wat[:, cols].reshape(CC, 128, COLS).astype(BF16_NP),
            "w_proj": Wpr[:, cols].reshape(CC, 128, COLS).astype(BF16_NP),
            "w_eps_proj": Wep[:, cols].reshape(CC, 128, COLS).astype(BF16_NP),
        })

    in_maps = []
    for core in range(NCORES):
        b, g = divmod(core, 2)
        in_maps.append({
            "xT_bf": xT_bf[b],
            "vT_bf": vT_bf[b],
            "cos_t": cosT,
            "sin_t": sinT,
            **per_g[g],
        })
    return in_maps


def _get_compiled():
    global _COMPILED
    if _COMPILED is None:
        _COMPILED = _build()
    return _COMPILED


def kernel(**inputs):
    nc = _get_compiled()
    in_maps = _prep_core_inputs(inputs)
    res = run_bass_kernel_spmd(nc, in_maps, list(range(NCORES)))
    v_out = np.empty((B, T, DIM), np.float32)
    x_out = np.empty((B, T, DIM), np.float32)
    for core in range(NCORES):
        b, g = divmod(core, 2)
        r = res.results[core]
        cols = slice(g * COLS, (g + 1) * COLS)
        v_out[b][:, cols] = r["v_out"].reshape(T, COLS).astype(np.float32)
        x_out[b][:, cols] = r["x_out"].reshape(T, COLS).astype(np.float32)
    return (v_out, x_out)


# revision 58
# speedup vs baseline: 1.0121x; 1.0017x over previous
"""Distributed Trainium2 (Bass/Tile) kernel for a causal self-attention block.

Reference computation (per batch b):
    qk = x_eps @ W_eps_attn ; q,k = split(qk) ; vp = v @ W_attn
    q,k = rope(q), rope(k)   (llama-style, 16 heads x 128 dims)
    y   = causal_softmax(q k^T / sqrt(128)) @ vp   (per head)
    v_out     = y @ W_proj
    x_eps_out = x_eps @ W_eps_proj

Sharding over 8 NeuronCores: core = (b, g) = 4-way batch x 2-way head-group
(8 heads per core).  W_eps_attn/W_attn are column-sharded by head; y is
exchanged pair-wise in two chunks (AllGather per 512-token half, pipelined
behind later compute) and W_proj/W_eps_proj are used column-sharded so each
core produces a disjoint half of both outputs (no reduce needed).

All matmuls run in bf16 with fp32 PSUM accumulation.  Activations are
uploaded contraction-major (pre-transposed host-side, numerically identical
to a device-side transpose of the same bf16 data); the rotate-half RoPE
layout, the 1/sqrt(128) q-scale and all weight tiling/permutation are pure
host-side weight re-layout.

Engine balance (v2): RoPE multiplies read the PSUM directly (no DVE
pre-copies), the causal mask is applied in-place on GpSimd via
affine_select, the softmax denominator uses the fast approximate DVE
reciprocal, vp/out PSUM evacuation is split between ScalarE and DVE, and
the x_eps_out projection is interleaved into the second attention half so
TensorE never waits on ScalarE's exp stream.
"""

import sys

sys.path.insert(0, "/opt/trn_rl_repo")

import numpy as np
import ml_dtypes

import concourse.bass as bass
import concourse.mybir as mybir
import concourse.tile as tile
from concourse import bacc
from concourse.bass_utils import run_bass_kernel_spmd

F32 = mybir.dt.float32
BF16 = mybir.dt.bfloat16
BF16_NP = ml_dtypes.bfloat16

B, T, DIM, H, HD = 4, 1024, 2048, 16, 128
NCORES = 8
HL = H // 2          # heads per core (8)
TT = T // 128        # t-tiles (8)
CC = DIM // 128      # contraction chunks (16)
QC = T // 512        # 512-wide q chunks (2)
COLS = DIM // 2      # local column count of each output half (1024)

_COMPILED = None
STAGE_MARKS = []


def _mark(nc, name):
    STAGE_MARKS.append((name, len(nc.inst_map)))


def _build():
    nc = bacc.Bacc(trn_type="TRN2", target_bir_lowering=False, debug=False,
                   num_devices=NCORES)

    # ---- per-core I/O (activations contraction-major, weights pre-packed) ----
    x_in = nc.dram_tensor("xT_bf", [CC, 128, T], BF16, kind="ExternalInput").ap()
    v_in = nc.dram_tensor("vT_bf", [CC, 128, T], BF16, kind="ExternalInput").ap()
    cos_in = nc.dram_tensor("cos_t", [64, T], BF16, kind="ExternalInput").ap()
    sin_in = nc.dram_tensor("sin_t", [64, T], BF16, kind="ExternalInput").ap()
    wqk_in = nc.dram_tensor("w_qk", [8, CC, 128, 256], BF16, kind="ExternalInput").ap()
    wat_in = nc.dram_tensor("w_attn", [CC, 128, COLS], BF16, kind="ExternalInput").ap()
    wpr_in = nc.dram_tensor("w_proj", [CC, 128, COLS], BF16, kind="ExternalInput").ap()
    wep_in = nc.dram_tensor("w_eps_proj", [CC, 128, COLS], BF16,
                            kind="ExternalInput").ap()
    v_out = nc.dram_tensor("v_out", [TT, 128, COLS], BF16, kind="ExternalOutput").ap()
    x_out = nc.dram_tensor("x_out", [TT, 128, COLS], BF16, kind="ExternalOutput").ap()

    # internal DRAM for the chunked pair-wise y exchange.  Half 1 is
    # exchanged in two 4-head sub-collectives so the first dispatches
    # mid-D1 and its gather half is resident well before V1 needs it.
    y_bounce = [nc.dram_tensor(f"y_bounce{qc}", [HL, 128, 512], BF16)
                for qc in range(QC)]
    y_gather = [nc.dram_tensor(f"y_gather{qc}", [2 * HL, 128, 512], BF16)
                for qc in range(QC)]
    y_gather1a = nc.dram_tensor("y_gather1a", [HL, 128, 512], BF16)
    y_gather1b = nc.dram_tensor("y_gather1b", [HL, 128, 512], BF16)

    with tile.TileContext(nc) as tc:
        with tc.tile_pool(name="persist", bufs=1) as pp, \
             tc.tile_pool(name="wstream", bufs=4) as wsp, \
             tc.tile_pool(name="ptile", bufs=6) as ptp, \
             tc.tile_pool(name="ropetmp", bufs=2) as rtp, \
             tc.tile_pool(name="outcp", bufs=3) as ocp, \
             tc.tile_pool(name="ps", bufs=6, space="PSUM") as psp:
            # PSUM budget: tag "ps" 6 banks (B groups, vp/proj subgroups,
            # score scratch) + tag "pyz" 2 banks (attention accumulators)

            # all-ones stationary operand: the denominator matmul then lands
            # Z replicated on every PSUM partition (free row-broadcast)
            ones_mat = pp.tile([128, 128], BF16, tag="ones_mat")
            nc.vector.memset(ones_mat[:], 1.0)
            # PE pre-warm: ~4.5us of dummy matmuls while the first xT/wqk
            # DMAs are in flight, so the HAM clock gate reaches 8/8 before
            # stage B's first real matmul (cold MMs run at 1.2 GHz)
            warm_ps = psp.tile([128, 128], F32, tag="ps", name="warm_ps")
            for _ in range(40):
                nc.tensor.matmul(warm_ps[:], ones_mat[:],
                                 ones_mat[:], start=True, stop=True)
            # cos/sin ride the scalar-engine DMA queue so the sync queue's
            # head of line is the stage-B critical path (xT[0] + wqk[0])
            cosT = pp.tile([64, T], BF16, tag="cosT")
            nc.scalar.dma_start(cosT[:], cos_in)
            sinT = pp.tile([64, T], BF16, tag="sinT")
            nc.scalar.dma_start(sinT[:], sin_in)
            # causal 0/1 masks, variant m: keep (1) iff q_rel - k_rel - 128*m
            # >= 0.  Applied as a DVE multiply: the gpsimd queue must stay
            # clear for the AllGather's DMA burst (it head-of-line blocks).
            masks = []
            for m in range(4):
                mk = pp.tile([128, 512], BF16, tag=f"mask{m}")
                nc.gpsimd.memset(mk[:], 1.0)
                nc.gpsimd.affine_select(
                    out=mk[:], in_=mk[:], compare_op=mybir.AluOpType.is_ge,
                    fill=0.0, base=-128 * m, pattern=[[1, 512]],
                    channel_multiplier=-1)
                masks.append(mk)

            xT = [pp.tile([128, T], BF16, tag=f"xT{c}", name=f"xT{c}")
                  for c in range(CC)]
            # v_out projection weights are made resident early so the final
            # (collective-dependent) stage never waits on a weight stream
            wprT = [pp.tile([128, COLS], BF16, tag=f"wprT{c}", name=f"wprT{c}")
                    for c in range(CC)]

            with tc.tile_pool(name="vtpool", bufs=1) as vtp:
                vT = [vtp.tile([128, T], BF16, tag=f"vT{c}", name=f"vT{c}")
                      for c in range(CC)]

                with tc.tile_pool(name="qkpool", bufs=1) as qkp:
                    qT = [qkp.tile([128, T], BF16, tag=f"qT{j}", name=f"qT{j}")
                          for j in range(HL)]
                    kT = [qkp.tile([128, T], BF16, tag=f"kT{j}", name=f"kT{j}")
                          for j in range(HL)]
                    vp = [qkp.tile([128, COLS], BF16, tag=f"vp{t}", name=f"vp{t}")
                          for t in range(TT)]

                    # ---- stage B: q/k projection (transposed out) + RoPE ----
                    # 8 groups of 2 d-tiles -> 4 live PSUM accumulators/group.
                    # xT loads are interleaved with group 0's weight stream so
                    # TensorE starts ~immediately.  RoPE reads the PSUM halves
                    # directly (DVE TensorTensor with one PSUM operand).
                    for G in range(8):
                        _mark(nc, f'B{G}')
                        # odd groups borrow the (B-phase-idle) pyz ring for
                        # their dl=1 accumulators so consecutive groups are
                        # fully double-buffered with ps=6 + pyz=2 banks
                        psums = [[psp.tile([128, 512], F32,
                                           tag=("pyz" if (G % 2 and i) else "ps"),
                                           bufs=(2 if (G % 2 and i) else None),
                                           name=f"psB{G}_{i}_{q}")
                                  for q in range(QC)] for i in range(2)]
                        for c in range(CC):
                            if G == 0:
                                nc.sync.dma_start(xT[c][:], x_in[c])
                            wt = wsp.tile([128, 256], BF16, tag="wqk_s",
                                          bufs=8)
                            nc.sync.dma_start(wt[:], wqk_in[G, c])
                            for dl in range(2):
                                for qc in range(QC):
                                    nc.tensor.matmul(
                                        psums[dl][qc][:],
                                        wt[:, dl * 128:(dl + 1) * 128],
                                        xT[c][:, qc * 512:(qc + 1) * 512],
                                        start=(c == 0), stop=(c == CC - 1))
                        for dl in range(2):
                            dt = 2 * G + dl
                            dst = qT[dt] if dt < HL else kT[dt - HL]
                            for qc in range(QC):
                                ps = psums[dl][qc]
                                cs = slice(qc * 512, (qc + 1) * 512)
                                # 4 DVE mults read the PSUM halves directly
                                # (frees the bank after the 4th); the two
                                # combines run on the otherwise-idle GpSimd
                                t1 = rtp.tile([64, 512], BF16, tag="rt1")
                                nc.vector.tensor_tensor(
                                    t1[:], ps[0:64, :], cosT[:, cs],
                                    mybir.AluOpType.mult)
                                t2 = rtp.tile([64, 512], BF16, tag="rt2")
                                nc.vector.tensor_tensor(
                                    t2[:], ps[64:128, :], sinT[:, cs],
                                    mybir.AluOpType.mult)
                                t3 = rtp.tile([64, 512], BF16, tag="rt3")
                                nc.vector.tensor_tensor(
                                    t3[:], ps[0:64, :], sinT[:, cs],
                                    mybir.AluOpType.mult)
                                t4 = rtp.tile([64, 512], BF16, tag="rt4")
                                nc.vector.tensor_tensor(
                                    t4[:], ps[64:128, :], cosT[:, cs],
                                    mybir.AluOpType.mult)
                                nc.gpsimd.tensor_tensor(
                                    dst[0:64, cs], t1[:], t2[:],
                                    mybir.AluOpType.subtract)
                                nc.gpsimd.tensor_tensor(
                                    dst[64:128, cs], t3[:], t4[:],
                                    mybir.AluOpType.add)

                    # vT arrives while stage B computes
                    for c in range(CC):
                        nc.sync.dma_start(vT[c][:], v_in[c])
                    for c in range(CC):
                        nc.sync.dma_start(wprT[c][:], wpr_in[c])

                    def vp_group(tg, tlist, copy_engine, pyz2=False):
                        # vp[t] for t in tlist: 16-chunk contraction.  pyz2
                        # puts the 2nd tile's accumulators in the pyz ring so
                        # back-to-back subgroups double-buffer within 6+2 banks
                        _mark(nc, f'C{tg}')
                        last = len(tlist) - 1
                        psums = [[psp.tile([128, 512], F32,
                                           tag=("pyz" if (pyz2 and i == last)
                                                else "ps"),
                                           bufs=(2 if (pyz2 and i == last)
                                                 else None),
                                           name=f"psC{t}_{q}")
                                  for q in range(2)] for i, t in enumerate(tlist)]
                        for c in range(CC):
                            wt = wsp.tile([128, COLS], BF16, tag="wat_s",
                                          bufs=6)
                            nc.sync.dma_start(wt[:], wat_in[c])
                            for i, t in enumerate(tlist):
                                for dh in range(2):
                                    nc.tensor.matmul(
                                        psums[i][dh][:],
                                        vT[c][:, t * 128:(t + 1) * 128],
                                        wt[:, dh * 512:(dh + 1) * 512],
                                        start=(c == 0), stop=(c == CC - 1))
                        for i, t in enumerate(tlist):
                            for dh in range(2):
                                eng = copy_engine[(i * 2 + dh) % len(copy_engine)]
                                if eng == 's':
                                    nc.scalar.copy(
                                        vp[t][:, dh * 512:(dh + 1) * 512],
                                        psums[i][dh][:])
                                else:
                                    nc.vector.tensor_copy(
                                        vp[t][:, dh * 512:(dh + 1) * 512],
                                        psums[i][dh][:])

                    def attn_head(j, qc):
                        # one head's attention for token half qc.  py/pz live
                        # in their own 2-bank ring so the next head's score
                        # tiles never wait on this head's normalization.
                        ktiles = 4 * qc + 4
                        py = psp.tile([128, 512], F32, tag="pyz", bufs=2,
                                      name=f"py{j}_{qc}")
                        pz = psp.tile([128, 512], F32, tag="pyz", bufs=2,
                                      name=f"pz{j}_{qc}")
                        for ki in range(ktiles):
                            pscr = psp.tile([128, 512], F32, tag="ps",
                                            name=f"pscr{j}_{qc}_{ki}")
                            nc.tensor.matmul(
                                pscr[:],
                                kT[j][:, ki * 128:(ki + 1) * 128],
                                qT[j][:, qc * 512:(qc + 1) * 512],
                                start=True, stop=True)
                            pt = ptp.tile([128, 512], BF16, tag="p_t")
                            nc.scalar.activation(
                                pt[:], pscr[:],
                                mybir.ActivationFunctionType.Exp)
                            m = ki - 4 * qc
                            if m >= 0:
                                nc.vector.tensor_tensor(
                                    pt[:], pt[:], masks[m][:],
                                    mybir.AluOpType.mult)
                            nc.tensor.matmul(
                                py[:], vp[ki][:, j * 128:(j + 1) * 128], pt[:],
                                start=(ki == 0), stop=(ki == ktiles - 1))
                            nc.tensor.matmul(
                                pz[:], ones_mat[:], pt[:],
                                start=(ki == 0), stop=(ki == ktiles - 1))
                        zr = rtp.tile([128, 512], F32, tag="zrec")
                        nc.vector.reciprocal_approx_fast(zr[:], pz[:])
                        yt = ptp.tile([128, 512], BF16, tag="y_t")
                        nc.vector.tensor_tensor(
                            yt[:], py[:], zr[:], mybir.AluOpType.mult)
                        nc.sync.dma_start(y_bounce[qc].ap()[j], yt[:])

                    def proj(w_of, lhs_fn, dst, tlist, psname, copy_engine,
                             pyz2=False):
                        # out[t, :] += lhs[c]^T @ w[c] over 16 chunks
                        _mark(nc, f'P{psname}')
                        last = len(tlist) - 1
                        psums = [[psp.tile([128, 512], F32,
                                           tag=("pyz" if (pyz2 and i == last)
                                                else "ps"),
                                           bufs=(2 if (pyz2 and i == last)
                                                 else None),
                                           name=f"ps{psname}_{t}_{q}")
                                  for q in range(2)] for i, t in enumerate(tlist)]
                        for c in range(CC):
                            wt = w_of(c)
                            for i, t in enumerate(tlist):
                                for dh in range(2):
                                    nc.tensor.matmul(
                                        psums[i][dh][:],
                                        lhs_fn(c, t),
                                        wt[:, dh * 512:(dh + 1) * 512],
                                        start=(c == 0), stop=(c == CC - 1))
                        for i, t in enumerate(tlist):
                            ot = ocp.tile([128, COLS], BF16, tag="out_cp")
                            for dh in range(2):
                                eng = copy_engine[(i * 2 + dh) % len(copy_engine)]
                                if eng == 's':
                                    nc.scalar.copy(
                                        ot[:, dh * 512:(dh + 1) * 512],
                                        psums[i][dh][:])
                                else:
                                    nc.vector.tensor_copy(
                                        ot[:, dh * 512:(dh + 1) * 512],
                                        psums[i][dh][:])
                            nc.sync.dma_start(dst[t], ot[:])

                    def wep_stream(c):
                        wt = wsp.tile([128, COLS], BF16, tag="wo_s")
                        nc.sync.dma_start(wt[:], wep_in[c])
                        return wt

                    xT_lhs = lambda c, t: xT[c][:, t * 128:(t + 1) * 128]

                    # ---- C0: vp tiles 0-3 (scalar evacuates; no exp nearby).
                    # Full 4-tile group: tiles 0-2 in the ps ring, tile 3
                    # borrows pyz, so W_attn streams once per half ----
                    vp_group(0, [0, 1, 2, 3], ['s', 'v'], pyz2=True)

                    # preload the Exp activation table off D0's critical path
                    # (the first Exp after C0's Copy activations reloads it)
                    dume = ptp.tile([1, 16], BF16, tag="dume")
                    nc.scalar.activation(dume[:], ones_mat[0:1, 0:16],
                                         mybir.ActivationFunctionType.Exp)

                    # ---- D0 (attention half 0) ----
                    _mark(nc, 'D0')
                    for j in range(HL):
                        attn_head(j, 0)
                    nc.gpsimd.collective_compute(
                        "AllGather", mybir.AluOpType.bypass,
                        replica_groups=[[0, 1], [2, 3], [4, 5], [6, 7]],
                        ins=[y_bounce[0].ap()], outs=[y_gather[0].ap()])

                    # ---- C1: vp tiles 4-7 (DVE evacuates; ScalarE owns Exp) --
                    vp_group(1, [4, 5, 6, 7], ['v'], pyz2=True)

                    # ---- D1 (attention half 1) with x_eps_out interleaved so
                    # TensorE has slack work while ScalarE streams exps ----
                    _mark(nc, 'D1')
                    attn_head(0, 1)
                    attn_head(1, 1)
                    proj(wep_stream, xT_lhs, x_out, [0, 1], "X0a", ['v'])
                    attn_head(2, 1)
                    attn_head(3, 1)
                    nc.gpsimd.collective_compute(
                        "AllGather", mybir.AluOpType.bypass,
                        replica_groups=[[0, 1], [2, 3], [4, 5], [6, 7]],
                        ins=[y_bounce[1].ap()[0:4]], outs=[y_gather1a.ap()])
                    proj(wep_stream, xT_lhs, x_out, [2, 3], "X0b", ['v'])
                    attn_head(4, 1)
                    attn_head(5, 1)
                    proj(wep_stream, xT_lhs, x_out, [4, 5], "X1a", ['v'])
                    attn_head(6, 1)
                    attn_head(7, 1)
                    nc.gpsimd.collective_compute(
                        "AllGather", mybir.AluOpType.bypass,
                        replica_groups=[[0, 1], [2, 3], [4, 5], [6, 7]],
                        ins=[y_bounce[1].ap()[4:8]], outs=[y_gather1b.ap()])

            # ---- tail: v_out projections + remaining x_eps_out ----
            # v_out rows [qc*512, ..+512) consume y columns of the same token
            # range, so group qc only needs exchange chunk qc.
            with tc.tile_pool(name="ygpool", bufs=1) as ygp, \
                 tc.tile_pool(name="wepx", bufs=1) as wxp:
                yG = [[ygp.tile([128, 512], BF16, tag=f"yG{qc}_{c}",
                                name=f"yG{qc}_{c}") for c in range(CC)]
                      for qc in range(QC)]
                # X1b's W_eps_proj chunks become resident in the SBUF space
                # vacated by vT/qT/kT/vp, streamed during V0 (no tail stall)
                wepx = [wxp.tile([128, COLS], BF16, tag=f"wepx{c}",
                                 name=f"wepx{c}") for c in range(CC)]
                for c in range(CC):
                    nc.sync.dma_start(wepx[c][:], wep_in[c])
                yG_lhs = lambda c, t: yG[t // 4][c][:, (t % 4) * 128:
                                                   (t % 4) * 128 + 128]
                # both gather halves prefetch on the gpsimd queue as soon as
                # their AllGather lands (yG[0] during late D1, yG[1] during V0)
                # both gather halves prefetch on the gpsimd queue as soon as
                # their AllGather lands (yG[0] during late D1, yG[1] during V0)
                def yg1_src(c):
                    # global-head chunk c -> sub-gather block (even-rank
                    # heads 0-7 are global 0-7, odd-rank heads are 8-15)
                    if c < 4:
                        return y_gather1a.ap()[c]
                    if c < 8:
                        return y_gather1b.ap()[c - 4]
                    if c < 12:
                        return y_gather1a.ap()[c - 4]
                    return y_gather1b.ap()[c - 8]

                for c in range(CC):
                    nc.gpsimd.dma_start(yG[0][c][:], y_gather[0].ap()[c])
                for c in [0, 1, 2, 3, 8, 9, 10, 11]:
                    nc.gpsimd.dma_start(yG[1][c][:], yg1_src(c))
                for c in [4, 5, 6, 7, 12, 13, 14, 15]:
                    nc.gpsimd.dma_start(yG[1][c][:], yg1_src(c))
                proj(lambda c: wprT[c], yG_lhs, v_out, [0, 1], "V0a",
                     ['s', 'v'])
                proj(lambda c: wprT[c], yG_lhs, v_out, [2, 3], "V0b",
                     ['s', 'v'], pyz2=True)
                proj(lambda c: wepx[c], xT_lhs, x_out, [6, 7], "X1b",
                     ['s', 'v'])
                proj(lambda c: wprT[c], yG_lhs, v_out, [4, 5], "V1a",
                     ['s', 'v'], pyz2=True)
                proj(lambda c: wprT[c], yG_lhs, v_out, [6, 7], "V1b",
                     ['s', 'v'])

    nc.compile()
    return nc


def _prep_core_inputs(inputs):
    """Host-side shard prep: slicing, bf16 cast, weight permutation/packing."""
    x_eps = np.asarray(inputs["x_eps"], np.float32)
    v = np.asarray(inputs["v"], np.float32)
    cos = np.asarray(inputs["freqs_cos"], np.float32)
    sin = np.asarray(inputs["freqs_sin"], np.float32)
    Wqk = np.asarray(inputs["W_eps_attn"], np.float32)
    Wat = np.asarray(inputs["W_attn"], np.float32)
    Wpr = np.asarray(inputs["W_proj"], np.float32)
    Wep = np.asarray(inputs["W_eps_proj"], np.float32)

    cosT = np.ascontiguousarray(cos.T).astype(BF16_NP)     # [64, T]
    sinT = np.ascontiguousarray(sin.T).astype(BF16_NP)
    perm = np.concatenate([np.arange(0, HD, 2), np.arange(1, HD, 2)])  # rotate-half
    scale = np.float32(1.0 / np.sqrt(HD))

    # contraction-major (transposed) bf16 activations, tiled [CC, 128, T]
    xT_bf = [np.ascontiguousarray(x_eps[b].astype(BF16_NP).T)
             .reshape(CC, 128, T) for b in range(B)]
    vT_bf = [np.ascontiguousarray(v[b].astype(BF16_NP).T)
             .reshape(CC, 128, T) for b in range(B)]

    per_g = []
    for g in range(2):
        heads = range(g * HL, (g + 1) * HL)
        wq = np.concatenate(
            [Wqk[:, h * HD:(h + 1) * HD][:, perm] * scale for h in heads], axis=1)
        wk = np.concatenate(
            [Wqk[:, DIM + h * HD:DIM + (h + 1) * HD][:, perm] for h in heads],
            axis=1)
        wqk_cols = np.concatenate([wq, wk], axis=1)          # [2048, 2048]
        wqk_packed = np.empty((8, CC, 128, 256), np.float32)
        for G in range(8):
            for dl in range(2):
                dt = 2 * G + dl
                blk = wqk_cols[:, dt * 128:(dt + 1) * 128]    # [2048, 128]
                wqk_packed[G, :, :, dl * 128:(dl + 1) * 128] = \
                    blk.reshape(CC, 128, 128)
        cols = slice(g * COLS, (g + 1) * COLS)
        per_g.append({
            "w_qk": wqk_packed.astype(BF16_NP),
            "w_attn": Wat[:, cols].reshape(CC, 128, COLS).astype(BF16_NP),
            "w_proj": Wpr[:, cols].reshape(CC, 128, COLS).astype(BF16_NP),
            "w_eps_proj": Wep[:, cols].reshape(CC, 128, COLS).astype(BF16_NP),
        })

    in_maps = []
    for core in range(NCORES):
        b, g = divmod(core, 2)
        in_maps.append({
            "xT_bf": xT_bf[b],
            "vT_bf": vT_bf[b],
            "cos_t": cosT,
            "sin_t": sinT,
            **per_g[g],
        })
    return in_maps


def _get_compiled():
    global _COMPILED
    if _COMPILED is None:
        _COMPILED = _build()
    return _COMPILED


def kernel(**inputs):
    nc = _get_compiled()
    in_maps = _prep_core_inputs(inputs)
    res = run_bass_kernel_spmd(nc, in_maps, list(range(NCORES)))
    v_out = np.empty((B, T, DIM), np.float32)
    x_out = np.empty((B, T, DIM), np.float32)
    for core in range(NCORES):
        b, g = divmod(core, 2)
        r = res.results[core]
        cols = slice(g * COLS, (g + 1) * COLS)
        v_out[b][:, cols] = r["v_out"].reshape(T, COLS).astype(np.float32)
        x_out[b][:, cols] = r["x_out"].reshape(T, COLS).astype(np.float32)
    return (v_out, x_out)


# revision 64
# speedup vs baseline: 1.0289x; 1.0166x over previous
"""Distributed Trainium2 (Bass/Tile) kernel for a causal self-attention block.

Reference computation (per batch b):
    qk = x_eps @ W_eps_attn ; q,k = split(qk) ; vp = v @ W_attn
    q,k = rope(q), rope(k)   (llama-style, 16 heads x 128 dims)
    y   = causal_softmax(q k^T / sqrt(128)) @ vp   (per head)
    v_out     = y @ W_proj
    x_eps_out = x_eps @ W_eps_proj

Sharding over 8 NeuronCores: core = (b, g) = 4-way batch x 2-way head-group
(8 heads per core).  W_eps_attn/W_attn are column-sharded by head; y is
exchanged pair-wise in two chunks (AllGather per 512-token half, pipelined
behind later compute) and W_proj/W_eps_proj are used column-sharded so each
core produces a disjoint half of both outputs (no reduce needed).

All matmuls run in bf16 with fp32 PSUM accumulation.  Activations are
uploaded contraction-major (pre-transposed host-side, numerically identical
to a device-side transpose of the same bf16 data); the rotate-half RoPE
layout, the 1/sqrt(128) q-scale and all weight tiling/permutation are pure
host-side weight re-layout.

Engine balance (v2): RoPE multiplies read the PSUM directly (no DVE
pre-copies), the causal mask is applied in-place on GpSimd via
affine_select, the softmax denominator uses the fast approximate DVE
reciprocal, vp/out PSUM evacuation is split between ScalarE and DVE, and
the x_eps_out projection is interleaved into the second attention half so
TensorE never waits on ScalarE's exp stream.
"""

import sys

sys.path.insert(0, "/opt/trn_rl_repo")

import numpy as np
import ml_dtypes

import concourse.bass as bass
import concourse.mybir as mybir
import concourse.tile as tile
from concourse import bacc
from concourse.bass_utils import run_bass_kernel_spmd

F32 = mybir.dt.float32
BF16 = mybir.dt.bfloat16
BF16_NP = ml_dtypes.bfloat16

B, T, DIM, H, HD = 4, 1024, 2048, 16, 128
NCORES = 8
HL = H // 2          # heads per core (8)
TT = T // 128        # t-tiles (8)
CC = DIM // 128      # contraction chunks (16)
QC = T // 512        # 512-wide q chunks (2)
COLS = DIM // 2      # local column count of each output half (1024)

_COMPILED = None
STAGE_MARKS = []


def _mark(nc, name):
    STAGE_MARKS.append((name, len(nc.inst_map)))


def _build():
    nc = bacc.Bacc(trn_type="TRN2", target_bir_lowering=False, debug=False,
                   num_devices=NCORES)

    # ---- per-core I/O (activations contraction-major, weights pre-packed) ----
    x_in = nc.dram_tensor("xT_bf", [CC, 128, T], BF16, kind="ExternalInput").ap()
    v_in = nc.dram_tensor("vT_bf", [CC, 128, T], BF16, kind="ExternalInput").ap()
    cos_in = nc.dram_tensor("cos_t", [64, T], BF16, kind="ExternalInput").ap()
    sin_in = nc.dram_tensor("sin_t", [64, T], BF16, kind="ExternalInput").ap()
    wqk_in = nc.dram_tensor("w_qk", [8, CC, 128, 256], BF16, kind="ExternalInput").ap()
    wat_in = nc.dram_tensor("w_attn", [CC, 128, COLS], BF16, kind="ExternalInput").ap()
    wpr_in = nc.dram_tensor("w_proj", [CC, 128, COLS], BF16, kind="ExternalInput").ap()
    wep_in = nc.dram_tensor("w_eps_proj", [CC, 128, COLS], BF16,
                            kind="ExternalInput").ap()
    v_out = nc.dram_tensor("v_out", [TT, 128, COLS], BF16, kind="ExternalOutput").ap()
    x_out = nc.dram_tensor("x_out", [TT, 128, COLS], BF16, kind="ExternalOutput").ap()

    # internal DRAM for the chunked pair-wise y exchange
    y_bounce = [nc.dram_tensor(f"y_bounce{qc}", [HL, 128, 512], BF16)
                for qc in range(QC)]
    y_gather = [nc.dram_tensor(f"y_gather{qc}", [2 * HL, 128, 512], BF16)
                for qc in range(QC)]

    with tile.TileContext(nc) as tc:
        with tc.tile_pool(name="persist", bufs=1) as pp, \
             tc.tile_pool(name="wstream", bufs=4) as wsp, \
             tc.tile_pool(name="ptile", bufs=6) as ptp, \
             tc.tile_pool(name="ropetmp", bufs=2) as rtp, \
             tc.tile_pool(name="outcp", bufs=3) as ocp, \
             tc.tile_pool(name="ps", bufs=6, space="PSUM") as psp:
            # PSUM budget: tag "ps" 6 banks (B groups, vp/proj subgroups,
            # score scratch) + tag "pyz" 2 banks (attention accumulators)

            # all-ones stationary operand: the denominator matmul then lands
            # Z replicated on every PSUM partition (free row-broadcast)
            ones_mat = pp.tile([128, 128], BF16, tag="ones_mat")
            nc.vector.memset(ones_mat[:], 1.0)
            # PE pre-warm: ~4.5us of dummy matmuls while the first xT/wqk
            # DMAs are in flight, so the HAM clock gate reaches 8/8 before
            # stage B's first real matmul (cold MMs run at 1.2 GHz)
            warm_ps = psp.tile([128, 128], F32, tag="ps", name="warm_ps")
            for _ in range(40):
                nc.tensor.matmul(warm_ps[:], ones_mat[:],
                                 ones_mat[:], start=True, stop=True)
            # cos/sin ride the scalar-engine DMA queue so the sync queue's
            # head of line is the stage-B critical path (xT[0] + wqk[0])
            cosT = pp.tile([64, T], BF16, tag="cosT")
            nc.scalar.dma_start(cosT[:], cos_in)
            sinT = pp.tile([64, T], BF16, tag="sinT")
            nc.scalar.dma_start(sinT[:], sin_in)
            # causal 0/1 masks, variant m: keep (1) iff q_rel - k_rel - 128*m
            # >= 0.  Applied as a DVE multiply: the gpsimd queue must stay
            # clear for the AllGather's DMA burst (it head-of-line blocks).
            masks = []
            for m in range(4):
                mk = pp.tile([128, 512], BF16, tag=f"mask{m}")
                nc.gpsimd.memset(mk[:], 1.0)
                nc.gpsimd.affine_select(
                    out=mk[:], in_=mk[:], compare_op=mybir.AluOpType.is_ge,
                    fill=0.0, base=-128 * m, pattern=[[1, 512]],
                    channel_multiplier=-1)
                masks.append(mk)

            xT = [pp.tile([128, T], BF16, tag=f"xT{c}", name=f"xT{c}")
                  for c in range(CC)]
            # v_out projection weights are made resident early so the final
            # (collective-dependent) stage never waits on a weight stream
            wprT = [pp.tile([128, COLS], BF16, tag=f"wprT{c}", name=f"wprT{c}")
                    for c in range(CC)]

            with tc.tile_pool(name="vtpool", bufs=1) as vtp:
                vT = [vtp.tile([128, T], BF16, tag=f"vT{c}", name=f"vT{c}")
                      for c in range(CC)]

                with tc.tile_pool(name="qkpool", bufs=1) as qkp:
                    qT = [qkp.tile([128, T], BF16, tag=f"qT{j}", name=f"qT{j}")
                          for j in range(HL)]
                    kT = [qkp.tile([128, T], BF16, tag=f"kT{j}", name=f"kT{j}")
                          for j in range(HL)]
                    vp = [qkp.tile([128, COLS], BF16, tag=f"vp{t}", name=f"vp{t}")
                          for t in range(TT)]

                    # ---- stage B: q/k projection (transposed out) + RoPE ----
                    # 8 groups of 2 d-tiles -> 4 live PSUM accumulators/group.
                    # xT loads are interleaved with group 0's weight stream so
                    # TensorE starts ~immediately.  RoPE reads the PSUM halves
                    # directly (DVE TensorTensor with one PSUM operand).
                    for G in range(8):
                        _mark(nc, f'B{G}')
                        # odd groups borrow the (B-phase-idle) pyz ring for
                        # their dl=1 accumulators so consecutive groups are
                        # fully double-buffered with ps=6 + pyz=2 banks
                        psums = [[psp.tile([128, 512], F32,
                                           tag=("pyz" if (G % 2 and i) else "ps"),
                                           bufs=(2 if (G % 2 and i) else None),
                                           name=f"psB{G}_{i}_{q}")
                                  for q in range(QC)] for i in range(2)]
                        for c in range(CC):
                            if G == 0:
                                nc.sync.dma_start(xT[c][:], x_in[c])
                            wt = wsp.tile([128, 256], BF16, tag="wqk_s",
                                          bufs=8)
                            nc.sync.dma_start(wt[:], wqk_in[G, c])
                            for dl in range(2):
                                for qc in range(QC):
                                    nc.tensor.matmul(
                                        psums[dl][qc][:],
                                        wt[:, dl * 128:(dl + 1) * 128],
                                        xT[c][:, qc * 512:(qc + 1) * 512],
                                        start=(c == 0), stop=(c == CC - 1))
                        for dl in range(2):
                            dt = 2 * G + dl
                            dst = qT[dt] if dt < HL else kT[dt - HL]
                            for qc in range(QC):
                                ps = psums[dl][qc]
                                cs = slice(qc * 512, (qc + 1) * 512)
                                # 4 DVE mults read the PSUM halves directly
                                # (frees the bank after the 4th); the two
                                # combines run on the otherwise-idle GpSimd
                                t1 = rtp.tile([64, 512], BF16, tag="rt1")
                                nc.vector.tensor_tensor(
                                    t1[:], ps[0:64, :], cosT[:, cs],
                                    mybir.AluOpType.mult)
                                t2 = rtp.tile([64, 512], BF16, tag="rt2")
                                nc.vector.tensor_tensor(
                                    t2[:], ps[64:128, :], sinT[:, cs],
                                    mybir.AluOpType.mult)
                                t3 = rtp.tile([64, 512], BF16, tag="rt3")
                                nc.vector.tensor_tensor(
                                    t3[:], ps[0:64, :], sinT[:, cs],
                                    mybir.AluOpType.mult)
                                t4 = rtp.tile([64, 512], BF16, tag="rt4")
                                nc.vector.tensor_tensor(
                                    t4[:], ps[64:128, :], cosT[:, cs],
                                    mybir.AluOpType.mult)
                                nc.gpsimd.tensor_tensor(
                                    dst[0:64, cs], t1[:], t2[:],
                                    mybir.AluOpType.subtract)
                                nc.gpsimd.tensor_tensor(
                                    dst[64:128, cs], t3[:], t4[:],
                                    mybir.AluOpType.add)

                    # vT arrives while stage B computes
                    for c in range(CC):
                        nc.sync.dma_start(vT[c][:], v_in[c])

                    def vp_group(tg, tlist, copy_engine, pyz2=False):
                        # vp[t] for t in tlist: 16-chunk contraction.  pyz2
                        # puts the 2nd tile's accumulators in the pyz ring so
                        # back-to-back subgroups double-buffer within 6+2 banks
                        _mark(nc, f'C{tg}')
                        last = len(tlist) - 1
                        psums = [[psp.tile([128, 512], F32,
                                           tag=("pyz" if (pyz2 and i == last)
                                                else "ps"),
                                           bufs=(2 if (pyz2 and i == last)
                                                 else None),
                                           name=f"psC{t}_{q}")
                                  for q in range(2)] for i, t in enumerate(tlist)]
                        for c in range(CC):
                            wt = wsp.tile([128, COLS], BF16, tag="wat_s",
                                          bufs=6)
                            nc.sync.dma_start(wt[:], wat_in[c])
                            for i, t in enumerate(tlist):
                                for dh in range(2):
                                    nc.tensor.matmul(
                                        psums[i][dh][:],
                                        vT[c][:, t * 128:(t + 1) * 128],
                                        wt[:, dh * 512:(dh + 1) * 512],
                                        start=(c == 0), stop=(c == CC - 1))
                        for i, t in enumerate(tlist):
                            for dh in range(2):
                                eng = copy_engine[(i * 2 + dh) % len(copy_engine)]
                                if eng == 's':
                                    nc.scalar.copy(
                                        vp[t][:, dh * 512:(dh + 1) * 512],
                                        psums[i][dh][:])
                                else:
                                    nc.vector.tensor_copy(
                                        vp[t][:, dh * 512:(dh + 1) * 512],
                                        psums[i][dh][:])

                    def attn_head(j, qc):
                        # one head's attention for token half qc.  py/pz live
                        # in their own 2-bank ring so the next head's score
                        # tiles never wait on this head's normalization.
                        ktiles = 4 * qc + 4
                        py = psp.tile([128, 512], F32, tag="pyz", bufs=2,
                                      name=f"py{j}_{qc}")
                        pz = psp.tile([128, 512], F32, tag="pyz", bufs=2,
                                      name=f"pz{j}_{qc}")
                        for ki in range(ktiles):
                            pscr = psp.tile([128, 512], F32, tag="ps",
                                            name=f"pscr{j}_{qc}_{ki}")
                            nc.tensor.matmul(
                                pscr[:],
                                kT[j][:, ki * 128:(ki + 1) * 128],
                                qT[j][:, qc * 512:(qc + 1) * 512],
                                start=True, stop=True)
                            pt = ptp.tile([128, 512], BF16, tag="p_t")
                            nc.scalar.activation(
                                pt[:], pscr[:],
                                mybir.ActivationFunctionType.Exp)
                            m = ki - 4 * qc
                            if m >= 0:
                                nc.vector.tensor_tensor(
                                    pt[:], pt[:], masks[m][:],
                                    mybir.AluOpType.mult)
                            nc.tensor.matmul(
                                py[:], vp[ki][:, j * 128:(j + 1) * 128], pt[:],
                                start=(ki == 0), stop=(ki == ktiles - 1))
                            nc.tensor.matmul(
                                pz[:], ones_mat[:], pt[:],
                                start=(ki == 0), stop=(ki == ktiles - 1))
                        zr = rtp.tile([128, 512], F32, tag="zrec")
                        nc.vector.reciprocal_approx_fast(zr[:], pz[:])
                        yt = ptp.tile([128, 512], BF16, tag="y_t")
                        nc.vector.tensor_tensor(
                            yt[:], py[:], zr[:], mybir.AluOpType.mult)
                        nc.sync.dma_start(y_bounce[qc].ap()[j], yt[:])

                    def proj(w_of, lhs_fn, dst, tlist, psname, copy_engine,
                             pyz2=False):
                        # out[t, :] += lhs[c]^T @ w[c] over 16 chunks
                        _mark(nc, f'P{psname}')
                        last = len(tlist) - 1
                        psums = [[psp.tile([128, 512], F32,
                                           tag=("pyz" if (pyz2 and i == last)
                                                else "ps"),
                                           bufs=(2 if (pyz2 and i == last)
                                                 else None),
                                           name=f"ps{psname}_{t}_{q}")
                                  for q in range(2)] for i, t in enumerate(tlist)]
                        for c in range(CC):
                            wt = w_of(c)
                            for i, t in enumerate(tlist):
                                for dh in range(2):
                                    nc.tensor.matmul(
                                        psums[i][dh][:],
                                        lhs_fn(c, t),
                                        wt[:, dh * 512:(dh + 1) * 512],
                                        start=(c == 0), stop=(c == CC - 1))
                        for i, t in enumerate(tlist):
                            ot = ocp.tile([128, COLS], BF16, tag="out_cp")
                            for dh in range(2):
                                eng = copy_engine[(i * 2 + dh) % len(copy_engine)]
                                if eng == 's':
                                    nc.scalar.copy(
                                        ot[:, dh * 512:(dh + 1) * 512],
                                        psums[i][dh][:])
                                else:
                                    nc.vector.tensor_copy(
                                        ot[:, dh * 512:(dh + 1) * 512],
                                        psums[i][dh][:])
                            nc.sync.dma_start(dst[t], ot[:])

                    def wep_stream(c):
                        wt = wsp.tile([128, COLS], BF16, tag="wo_s")
                        nc.sync.dma_start(wt[:], wep_in[c])
                        return wt

                    xT_lhs = lambda c, t: xT[c][:, t * 128:(t + 1) * 128]

                    # ---- C0: vp tiles 0-3 (scalar evacuates; no exp nearby).
                    # Full 4-tile group: tiles 0-2 in the ps ring, tile 3
                    # borrows pyz, so W_attn streams once per half ----
                    vp_group(0, [0, 1, 2, 3], ['s', 'v'], pyz2=True)

                    # preload the Exp activation table off D0's critical path
                    # (the first Exp after C0's Copy activations reloads it)
                    dume = ptp.tile([1, 16], BF16, tag="dume")
                    nc.scalar.activation(dume[:], ones_mat[0:1, 0:16],
                                         mybir.ActivationFunctionType.Exp)

                    # ---- D0 (attention half 0) ----
                    _mark(nc, 'D0')
                    for j in range(HL):
                        attn_head(j, 0)
                    nc.gpsimd.collective_compute(
                        "AllGather", mybir.AluOpType.bypass,
                        replica_groups=[[0, 1], [2, 3], [4, 5], [6, 7]],
                        ins=[y_bounce[0].ap()], outs=[y_gather[0].ap()])

                    # ---- C1: vp tiles 4-7 (DVE evacuates; ScalarE owns Exp) --
                    vp_group(1, [4, 5, 6, 7], ['v'], pyz2=True)
                    # wprT lands during D1: B/C0's HBM stays dedicated to the
                    # wqk/xT/vT/wat streams
                    for c in range(CC):
                        nc.sync.dma_start(wprT[c][:], wpr_in[c])

                    # ---- D1 (attention half 1) with x_eps_out interleaved so
                    # TensorE has slack work while ScalarE streams exps ----
                    _mark(nc, 'D1')
                    attn_head(0, 1)
                    attn_head(1, 1)
                    proj(wep_stream, xT_lhs, x_out, [0, 1], "X0a", ['v'])
                    attn_head(2, 1)
                    attn_head(3, 1)
                    proj(wep_stream, xT_lhs, x_out, [2, 3], "X0b", ['v'])
                    attn_head(4, 1)
                    attn_head(5, 1)
                    proj(wep_stream, xT_lhs, x_out, [4, 5], "X1a", ['v'])
                    attn_head(6, 1)
                    attn_head(7, 1)
                    nc.gpsimd.collective_compute(
                        "AllGather", mybir.AluOpType.bypass,
                        replica_groups=[[0, 1], [2, 3], [4, 5], [6, 7]],
                        ins=[y_bounce[1].ap()], outs=[y_gather[1].ap()])

            # ---- tail: v_out projections + remaining x_eps_out ----
            # v_out rows [qc*512, ..+512) consume y columns of the same token
            # range, so group qc only needs exchange chunk qc.
            with tc.tile_pool(name="ygpool", bufs=1) as ygp, \
                 tc.tile_pool(name="wepx", bufs=1) as wxp:
                yG = [[ygp.tile([128, 512], BF16, tag=f"yG{qc}_{c}",
                                name=f"yG{qc}_{c}") for c in range(CC)]
                      for qc in range(QC)]
                # X1b's W_eps_proj chunks become resident in the SBUF space
                # vacated by vT/qT/kT/vp, streamed during V0 (no tail stall)
                wepx = [wxp.tile([128, COLS], BF16, tag=f"wepx{c}",
                                 name=f"wepx{c}") for c in range(CC)]
                for c in range(CC):
                    nc.sync.dma_start(wepx[c][:], wep_in[c])
                yG_lhs = lambda c, t: yG[t // 4][c][:, (t % 4) * 128:
                                                   (t % 4) * 128 + 128]
                # both gather halves prefetch on the gpsimd queue as soon as
                # their AllGather lands (yG[0] during late D1, yG[1] during V0)
                # both gather halves prefetch on the gpsimd queue as soon as
                # their AllGather lands (yG[0] during late D1, yG[1] during V0)
                for c in range(CC):
                    nc.gpsimd.dma_start(yG[0][c][:], y_gather[0].ap()[c])
                for c in range(CC):
                    nc.gpsimd.dma_start(yG[1][c][:], y_gather[1].ap()[c])
                proj(lambda c: wprT[c], yG_lhs, v_out, [0, 1], "V0a",
                     ['s', 'v'])
                proj(lambda c: wprT[c], yG_lhs, v_out, [2, 3], "V0b",
                     ['s', 'v'], pyz2=True)
                proj(lambda c: wepx[c], xT_lhs, x_out, [6, 7], "X1b",
                     ['s', 'v'])
                proj(lambda c: wprT[c], yG_lhs, v_out, [4, 5], "V1a",
                     ['s', 'v'], pyz2=True)
                proj(lambda c: wprT[c], yG_lhs, v_out, [6, 7], "V1b",
                     ['s', 'v'])

    nc.compile()
    return nc


def _prep_core_inputs(inputs):
    """Host-side shard prep: slicing, bf16 cast, weight permutation/packing."""
    x_eps = np.asarray(inputs["x_eps"], np.float32)
    v = np.asarray(inputs["v"], np.float32)
    cos = np.asarray(inputs["freqs_cos"], np.float32)
    sin = np.asarray(inputs["freqs_sin"], np.float32)
    Wqk = np.asarray(inputs["W_eps_attn"], np.float32)
    Wat = np.asarray(inputs["W_attn"], np.float32)
    Wpr = np.asarray(inputs["W_proj"], np.float32)
    Wep = np.asarray(inputs["W_eps_proj"], np.float32)

    cosT = np.ascontiguousarray(cos.T).astype(BF16_NP)     # [64, T]
    sinT = np.ascontiguousarray(sin.T).astype(BF16_NP)
    perm = np.concatenate([np.arange(0, HD, 2), np.arange(1, HD, 2)])  # rotate-half
    scale = np.float32(1.0 / np.sqrt(HD))

    # contraction-major (transposed) bf16 activations, tiled [CC, 128, T]
    xT_bf = [np.ascontiguousarray(x_eps[b].astype(BF16_NP).T)
             .reshape(CC, 128, T) for b in range(B)]
    vT_bf = [np.ascontiguousarray(v[b].astype(BF16_NP).T)
             .reshape(CC, 128, T) for b in range(B)]

    per_g = []
    for g in range(2):
        heads = range(g * HL, (g + 1) * HL)
        wq = np.concatenate(
            [Wqk[:, h * HD:(h + 1) * HD][:, perm] * scale for h in heads], axis=1)
        wk = np.concatenate(
            [Wqk[:, DIM + h * HD:DIM + (h + 1) * HD][:, perm] for h in heads],
            axis=1)
        wqk_cols = np.concatenate([wq, wk], axis=1)          # [2048, 2048]
        wqk_packed = np.empty((8, CC, 128, 256), np.float32)
        for G in range(8):
            for dl in range(2):
                dt = 2 * G + dl
                blk = wqk_cols[:, dt * 128:(dt + 1) * 128]    # [2048, 128]
                wqk_packed[G, :, :, dl * 128:(dl + 1) * 128] = \
                    blk.reshape(CC, 128, 128)
        cols = slice(g * COLS, (g + 1) * COLS)
        per_g.append({
            "w_qk": wqk_packed.astype(BF16_NP),
            "w_attn": Wat[:, cols].reshape(CC, 128, COLS).astype(BF16_NP),
            "w_proj": Wpr[:, cols].reshape(CC, 128, COLS).astype(BF16_NP),
            "w_eps_proj": Wep[:, cols].reshape(CC, 128, COLS).astype(BF16_NP),
        })

    in_maps = []
    for core in range(NCORES):
        b, g = divmod(core, 2)
        in_maps.append({
            "xT_bf": xT_bf[b],
            "vT_bf": vT_bf[b],
            "cos_t": cosT,
            "sin_t": sinT,
            **per_g[g],
        })
    return in_maps


def _get_compiled():
    global _COMPILED
    if _COMPILED is None:
        _COMPILED = _build()
    return _COMPILED


def kernel(**inputs):
    nc = _get_compiled()
    in_maps = _prep_core_inputs(inputs)
    res = run_bass_kernel_spmd(nc, in_maps, list(range(NCORES)))
    v_out = np.empty((B, T, DIM), np.float32)
    x_out = np.empty((B, T, DIM), np.float32)
    for core in range(NCORES):
        b, g = divmod(core, 2)
        r = res.results[core]
        cols = slice(g * COLS, (g + 1) * COLS)
        v_out[b][:, cols] = r["v_out"].reshape(T, COLS).astype(np.float32)
        x_out[b][:, cols] = r["x_out"].reshape(T, COLS).astype(np.float32)
    return (v_out, x_out)
